# revision 1
# baseline (speedup 1.0000x reference)
"""ConvGuidedFilter Trainium2 kernel (8-core SPMD, data parallel over (image, row-half)).

Shapes hardcoded for guide/src [4,3,1024,1024] f32, RADIUS=64, STRIDE=32 (box kernel
of ones). Each core handles one (image b, row-half h) shard: [3, 512, 1024].

Per core:
  P1: load shard; products g*s (DVE), g*g (ACT); 32-row block sums via PE matmuls
      (lhsT = 1/4096 block indicators, accumulated over the 4 row tiles) into PSUM
      [64, 1024] pairs (stats {g,s} at bases {0,32} of tile A, {gs,gg} in tile B);
      column-block reduce (DVE, segments of 32) -> [64,32]; col pair-sum -> [64,31].
  CC: AllGather of the per-core 5952-float stats payload across all 8 cores.
  P2 (replicated on all cores): assemble stats, row pair-sum, cov/var, 1x1-conv MLP
      with exact global-batch BatchNorm stats, produce A^T/B^T tiles [31j, (i,c)].
  P3: bilinear upsample via PE matmuls (col: Wcol const; row: per-core zero-padded
      Urow so the single SPMD program selects its own image);
      out = meanA*guide (DVE) + meanB (DVE evict + POOL add); DMA out.
"""

import numpy as np

import concourse.bass as bass
import concourse.bacc as bacc
import concourse.mybir as mybir
import concourse.tile as tile
from concourse.bass_utils import run_bass_kernel_spmd

AF = mybir.ActivationFunctionType
ALU = mybir.AluOpType
AX = mybir.AxisListType
F32 = mybir.dt.float32

B, C, H, W = 4, 3, 1024, 1024
NCORES = 8
SH = H // 2          # 512 rows per shard
NT = SH // 128       # 4 row tiles per channel
OB = 31              # box output spatial size
RBL = 16             # row blocks per shard (512/32)
PIX = OB * OB        # 961
NPIX = B * PIX       # 3844
EPS = 1e-5
PAYLOAD = C * 4 * RBL * OB   # 5952


def _build_nc(dump=False):
    nc = bacc.Bacc("TRN2", target_bir_lowering=False, debug=False, num_devices=NCORES)

    g_d = nc.dram_tensor("g_sh", [C, SH, W], F32, kind="ExternalInput")
    s_d = nc.dram_tensor("s_sh", [C, SH, W], F32, kind="ExternalInput")
    bsum_d = nc.dram_tensor("bsum", [128, 64], F32, kind="ExternalInput")
    pair_d = nc.dram_tensor("pairm", [32, OB], F32, kind="ExternalInput")
    wcol_d = nc.dram_tensor("wcol", [64, W], F32, kind="ExternalInput")
    urow0_d = nc.dram_tensor("urow0", [64, SH], F32, kind="ExternalInput")
    urow1_d = nc.dram_tensor("urow1", [64, SH], F32, kind="ExternalInput")
    w1ct_d = nc.dram_tensor("w1ct", [64, 32], F32, kind="ExternalInput")
    w1vt_d = nc.dram_tensor("w1vt", [64, 32], F32, kind="ExternalInput")
    w2t_d = nc.dram_tensor("w2t", [32, 32], F32, kind="ExternalInput")
    w3t_d = nc.dram_tensor("w3t", [32, 3], F32, kind="ExternalInput")
    bn_d = nc.dram_tensor("bn", [32, 5], F32, kind="ExternalInput")  # g1,b1,g2,b2,eps
    out_d = nc.dram_tensor("out_sh", [C, SH, W], F32, kind="ExternalOutput")
    if dump:
        dmp = {
            "pay": nc.dram_tensor("d_pay", [C, 2, 64, OB], F32, kind="ExternalOutput"),
            "sf": nc.dram_tensor("d_sf", [2, 64, 4 * 32 * OB], F32, kind="ExternalOutput"),
            "sf2": nc.dram_tensor("d_sf2", [32, B * 2 * C * OB], F32, kind="ExternalOutput"),
            "stat": nc.dram_tensor("d_stat", [2, 64, 4 * PIX], F32, kind="ExternalOutput"),
            "cov": nc.dram_tensor("d_cov", [2, 64, PIX], F32, kind="ExternalOutput"),
            "var": nc.dram_tensor("d_var", [2, 64, PIX], F32, kind="ExternalOutput"),
            "h1a": nc.dram_tensor("d_h1a", [32, NPIX], F32, kind="ExternalOutput"),
            "h2a": nc.dram_tensor("d_h2a", [32, NPIX], F32, kind="ExternalOutput"),
            "ajs": nc.dram_tensor("d_ajs", [2, 64, 3 * OB], F32, kind="ExternalOutput"),
            "bjs": nc.dram_tensor("d_bjs", [2, 64, 3 * OB], F32, kind="ExternalOutput"),
            "aws": nc.dram_tensor("d_aws", [2, 64, W], F32, kind="ExternalOutput"),
            "bws": nc.dram_tensor("d_bws", [2, 64, W], F32, kind="ExternalOutput"),
            "prd": nc.dram_tensor("d_prd", [128, 512], F32, kind="ExternalOutput"),
            "ccin": nc.dram_tensor("d_ccin", [C, 4, RBL, OB], F32, kind="ExternalOutput"),
            "ccout": nc.dram_tensor("d_ccout", [NCORES, PAYLOAD], F32, kind="ExternalOutput"),
            "mbs": nc.dram_tensor("d_mbs", [128, 512], F32, kind="ExternalOutput"),
        }

    with tile.TileContext(nc) as tc:
        with (
            tc.tile_pool(name="consts", bufs=1) as consts,
            tc.tile_pool(name="gres", bufs=1) as gres,
            tc.tile_pool(name="persist", bufs=1) as persist,
            tc.tile_pool(name="dram", bufs=1, space="DRAM") as dram,
        ):
            def const(name, dram_t, shape):
                t = consts.tile(shape, F32, tag=name, name=name + "_sb")
                nc.sync.dma_start(t[:], dram_t[:])
                return t

            bsum = const("bsum", bsum_d, [128, 64])
            pairm = const("pairm", pair_d, [32, OB])
            wcol = const("wcol", wcol_d, [64, W])
            urow = [const("urow0", urow0_d, [64, SH]),
                    const("urow1", urow1_d, [64, SH])]
            w1ct = const("w1ct", w1ct_d, [64, 32])
            w1vt = const("w1vt", w1vt_d, [64, 32])
            w2t = const("w2t", w2t_d, [32, 32])
            w3t = const("w3t", w3t_d, [32, 3])
            bn = const("bn", bn_d, [32, 5])

            # Guide shard kept resident in SBUF: [128, (c,t)*1024]
            G = gres.tile([128, C * NT * 1024], F32)

            PAYF = persist.tile([64, 2 * C * OB], F32)
            cc_in = dram.tile([64, 2 * C * OB], F32)
            cc_out = dram.tile([NCORES, 64, 2 * C * OB], F32)

            # ---------------- Phase 1 ----------------
            # Stats order s: 0=g(mean_x), 1=src(mean_y), 2=g*s, 3=g*g.
            # P tile A holds stats 0,1 at partition bases 0,32; tile B stats 2,3.
            with (
                tc.tile_pool(name="sload", bufs=3) as sload,
                tc.tile_pool(name="prod", bufs=2) as prod,
                tc.tile_pool(name="psum1", bufs=2, space="PSUM") as psum1,
                tc.tile_pool(name="cred", bufs=2) as cred,
            ):
                for c in range(C):
                    P = [psum1.tile([64, 1024], F32, tag=f"p1_{pi}",
                                    name=f"P{c}_{pi}") for pi in range(2)]
                    for t in range(NT):
                        gsl = G[:, (c * NT + t) * 1024:(c * NT + t + 1) * 1024]
                        nc.sync.dma_start(gsl, g_d[c, t * 128:(t + 1) * 128, :])
                        st = sload.tile([128, 1024], F32, tag="st")
                        nc.sync.dma_start(st[:], s_d[c, t * 128:(t + 1) * 128, :])
                        gs = prod.tile([128, 1024], F32, tag="gs")
                        nc.vector.tensor_mul(gs[:], gsl, st[:])
                        gg = prod.tile([128, 1024], F32, tag="gg")
                        nc.scalar.activation(gg[:], gsl, AF.Square)
                        for s_idx, srct in enumerate((gsl, st[:], gs[:], gg[:])):
                            pi, sl = s_idx // 2, s_idx % 2
                            for hh in range(2):
                                nc.tensor.matmul(
                                    P[pi][32 * sl:32 * sl + 16,
                                          hh * 512:hh * 512 + 512],
                                    bsum[:, 16 * t:16 * t + 16],
                                    srct[:, hh * 512:hh * 512 + 512],
                                    start=(t == 0), stop=(t == NT - 1),
                                )
                    for pi in range(2):
                        CR = cred.tile([64, 32], F32, tag=f"cr_{pi}")
                        nc.vector.tensor_reduce(
                            CR[:], P[pi][:].rearrange("p (a b) -> p a b", b=32),
                            axis=AX.X, op=ALU.add,
                        )
                        # stat (pi, sl) at partitions 32*sl+r; free pi*93+c*31+j
                        nc.vector.tensor_add(
                            PAYF[:, pi * 3 * OB + c * OB:pi * 3 * OB + (c + 1) * OB],
                            CR[:, 0:OB], CR[:, 1:32])
                if dump:
                    for pi in range(2):
                        pass
                nc.sync.dma_start(cc_in[:], PAYF[:])

            nc.gpsimd.collective_compute(
                "AllGather",
                ALU.bypass,
                replica_groups=[list(range(NCORES))],
                ins=[cc_in.opt()],
                outs=[cc_out.opt()],
            )

            # ---------------- Phase 2 ----------------
            with (
                tc.tile_pool(name="p2", bufs=1) as p2,
                tc.tile_pool(name="p2s", bufs=2) as p2s,
                tc.tile_pool(name="small", bufs=2) as small,
            ):
                # SFg: [64, (s,rb_g,j)]; image 2g+bl channels at base 32*bl
                SF = [p2.tile([64, 4 * 32 * OB], F32, name=f"SF{g}", tag="big",
                              bufs=3) for g in range(2)]
                for g in range(2):
                    for hh in range(2):
                        for pi in range(2):
                            for sl in range(2):
                                s = 2 * pi + sl
                                for bl in range(2):
                                    srcap = cc_out[:].rearrange(
                                        "(gg bl h) (ss r) (pp c j) -> gg h pp ss bl c r j",
                                        gg=2, bl=2, ss=2, pp=2, c=3)[
                                        g, hh, pi, sl, bl][:, 0:RBL, :]
                                    dst = SF[g][:].rearrange(
                                        "(bl cc) (s h r j) -> s h bl cc r j",
                                        bl=2, s=4, h=2, r=RBL)[s, hh, bl, 0:3]
                                    nc.sync.dma_start(dst, srcap)
                SF2 = p2.tile([32, B * 2 * C * OB], F32)  # [rb_g, (b, s2, c, j)]
                for hh in range(2):
                    for b in range(B):
                        for si in range(2):
                            srcap = cc_out[:].rearrange(
                                "(bb h) (ss r) (pp c j) -> bb h pp ss r c j",
                                bb=4, ss=2, pp=2, c=3)[b, hh, 0, si][0:RBL]
                            dst = SF2[16 * hh:16 * hh + RBL,
                                      (b * 2 + si) * 3 * OB:
                                      (b * 2 + si + 1) * 3 * OB].rearrange(
                                      "r (c j) -> r c j", c=3)
                            nc.sync.dma_start(dst, srcap)
                STAT, COV, VAR = [], [], []
                for g in range(2):
                    eng = nc.vector if g == 0 else nc.gpsimd
                    ST = p2.tile([64, 4 * PIX], F32, name=f"STAT{g}", tag="big",
                                 bufs=3)
                    sfr = SF[g][:].rearrange("p (s r j) -> p s r j", s=4, r=32)
                    eng.tensor_add(
                        ST[:].rearrange("p (s i j) -> p s i j", s=4, i=OB),
                        sfr[:, :, 0:OB, :], sfr[:, :, 1:32, :])
                    CVg = p2.tile([64, PIX], F32, name=f"COV{g}", tag="mid", bufs=5)
                    VRg = p2.tile([64, PIX], F32, name=f"VAR{g}", tag="mid", bufs=5)
                    TMg = p2.tile([64, PIX], F32, name=f"TMP{g}", tag="mid", bufs=5)
                    eng.tensor_mul(TMg[:], ST[:, 0:PIX], ST[:, PIX:2 * PIX])
                    eng.tensor_sub(CVg[:], ST[:, 2 * PIX:3 * PIX], TMg[:])
                    eng.tensor_mul(TMg[:], ST[:, 0:PIX], ST[:, 0:PIX])
                    eng.tensor_sub(VRg[:], ST[:, 3 * PIX:4 * PIX], TMg[:])
                    STAT.append(ST); COV.append(CVg); VAR.append(VRg)
                    if dump:
                        nc.sync.dma_start(dmp["stat"][g], ST[:])
                        nc.sync.dma_start(dmp["cov"][g], CVg[:])
                        nc.sync.dma_start(dmp["var"][g], VRg[:])

                chunks = [(0, 512), (512, PIX)]

                def bn_combine(SU, SQ, scn, tbn, gcol, bcol):
                    st_ = small.tile([32, 1], F32, tag="st_" + scn)
                    qt_ = small.tile([32, 1], F32, tag="qt_" + scn)
                    nc.vector.tensor_reduce(st_[:], SU[:], axis=AX.X, op=ALU.add)
                    nc.vector.tensor_reduce(qt_[:], SQ[:], axis=AX.X, op=ALU.add)
                    m_ = small.tile([32, 1], F32, tag="m_" + scn)
                    nc.vector.tensor_scalar_mul(m_[:], st_[:], 1.0 / NPIX)
                    v_ = small.tile([32, 1], F32, tag="v_" + scn)
                    nc.vector.tensor_scalar_mul(v_[:], qt_[:], 1.0 / NPIX)
                    mm_ = small.tile([32, 1], F32, tag="mm_" + scn)
                    nc.vector.tensor_mul(mm_[:], m_[:], m_[:])
                    nc.vector.tensor_sub(v_[:], v_[:], mm_[:])
                    sd_ = small.tile([32, 1], F32, tag="sd_" + scn)
                    nc.scalar.activation(sd_[:], v_[:], AF.Sqrt, bias=bn[:, 4:5])
                    rs_ = small.tile([32, 1], F32, tag="rs_" + scn)
                    nc.vector.reciprocal(rs_[:], sd_[:])
                    sc_ = small.tile([32, 1], F32, tag=scn)
                    nc.vector.tensor_mul(sc_[:], rs_[:], bn[:, gcol:gcol + 1])
                    tb_ = small.tile([32, 1], F32, tag=tbn)
                    nc.vector.tensor_mul(mm_[:], m_[:], sc_[:])
                    nc.vector.tensor_sub(tb_[:], bn[:, bcol:bcol + 1], mm_[:])
                    return sc_, tb_

                with tc.tile_pool(name="psumH", bufs=1, space="PSUM") as psumH:
                    # Layer 1
                    H1p = [psumH.tile([32, PIX], F32, tag=f"h_{b}", name=f"H1p{b}")
                           for b in range(B)]
                    S1 = p2s.tile([32, B], F32, tag="S1")
                    Q1 = p2s.tile([32, B], F32, tag="Q1")
                    SQs = p2s.tile([32, PIX], F32, tag="sqs")
                    for b in range(B):
                        g, bl = b // 2, b % 2
                        for (o, e) in chunks:
                            nc.tensor.matmul(H1p[b][:, o:e],
                                             w1ct[32 * bl:32 * bl + 3, :],
                                             COV[g][32 * bl:32 * bl + 3, o:e],
                                             start=True, stop=False)
                            nc.tensor.matmul(H1p[b][:, o:e],
                                             w1vt[32 * bl:32 * bl + 3, :],
                                             VAR[g][32 * bl:32 * bl + 3, o:e],
                                             start=False, stop=True)
                        nc.vector.tensor_reduce(S1[:, b:b + 1], H1p[b][:],
                                                axis=AX.X, op=ALU.add)
                        nc.scalar.activation(SQs[:], H1p[b][:], AF.Square,
                                             accum_out=Q1[:, b:b + 1])
                    sc1, tb1 = bn_combine(S1, Q1, "sc1", "tb1", 0, 1)
                    H1A = p2.tile([32, NPIX], F32, tag="big", bufs=3)
                    for b in range(B):
                        nc.scalar.activation(H1A[:, b * PIX:(b + 1) * PIX],
                                             H1p[b][:], AF.Relu,
                                             bias=tb1[:], scale=sc1[:])

                    # Layer 2
                    H2p = [psumH.tile([32, PIX], F32, tag=f"h_{b}", name=f"H2p{b}")
                           for b in range(B)]
                    S2 = p2s.tile([32, B], F32, tag="S2")
                    Q2 = p2s.tile([32, B], F32, tag="Q2")
                    for b in range(B):
                        for (o, e) in chunks:
                            nc.tensor.matmul(H2p[b][:, o:e], w2t[:],
                                             H1A[:, b * PIX + o:b * PIX + e],
                                             start=True, stop=True)
                        nc.vector.tensor_reduce(S2[:, b:b + 1], H2p[b][:],
                                                axis=AX.X, op=ALU.add)
                        nc.scalar.activation(SQs[:], H2p[b][:], AF.Square,
                                             accum_out=Q2[:, b:b + 1])
                    sc2, tb2 = bn_combine(S2, Q2, "sc2", "tb2", 2, 3)
                    H2A = p2.tile([32, NPIX], F32, tag="big", bufs=3)
                    for b in range(B):
                        nc.scalar.activation(H2A[:, b * PIX:(b + 1) * PIX],
                                             H2p[b][:], AF.Relu,
                                             bias=tb2[:], scale=sc2[:])

                if dump:
                    nc.sync.dma_start(dmp["h1a"][:], H1A[:])
                    nc.sync.dma_start(dmp["h2a"][:], H2A[:])
                # Transposed coefficient tiles: AJs/BJs group g holds images
                # {2g, 2g+1} at partition bases {0, 32}; layout [31j, (i,c)].
                AJs = [p2.tile([64, 3 * OB], F32, tag=f"ajs{g}", name=f"AJs{g}") for g in range(2)]
                BJs = [p2.tile([64, 3 * OB], F32, tag=f"bjs{g}", name=f"BJs{g}") for g in range(2)]
                with tc.tile_pool(name="psumT", bufs=2, space="PSUM") as psumT:
                    for g in range(2):
                        AJp = psumT.tile([64, 3 * OB], F32, tag="ajp")
                        MXJp = psumT.tile([64, 3 * OB], F32, tag="mxjp")
                        MYJp = psumT.tile([64, 3 * OB], F32, tag="myjp")
                        for bl in range(2):
                            b = 2 * g + bl
                            for i in range(OB):
                                nc.tensor.matmul(
                                    AJp[32 * bl:32 * bl + OB, 3 * i:3 * i + 3],
                                    H2A[:, b * PIX + OB * i:b * PIX + OB * i + OB],
                                    w3t[:], start=True, stop=True)
                            for si, MJp in ((0, MXJp), (1, MYJp)):
                                for c in range(C):
                                    o = (b * 2 * C + si * C + c) * OB
                                    nc.tensor.matmul(
                                        MJp[32 * bl:32 * bl + OB, :].rearrange(
                                            "p (i c) -> p c i", c=3)[:, c, :],
                                        SF2[:, o:o + OB],
                                        pairm[:], start=True, stop=True)
                        nc.scalar.copy(AJs[g][:], AJp[:])
                        MYs = p2s.tile([64, 3 * OB], F32, tag="mys")
                        nc.scalar.copy(MYs[:], MYJp[:])
                        TJ = p2s.tile([64, 3 * OB], F32, tag="tj")
                        nc.vector.tensor_mul(TJ[:], AJs[g][:], MXJp[:])
                        nc.vector.tensor_sub(BJs[g][:], MYs[:], TJ[:])
                        if dump:
                            nc.sync.dma_start(dmp["ajs"][g], AJs[g][:])
                            nc.sync.dma_start(dmp["bjs"][g], BJs[g][:])

                # ---------------- Phase 3 ----------------
                with (
                    tc.tile_pool(name="psumU", bufs=2, space="PSUM") as psumU,
                    tc.tile_pool(name="psumM", bufs=4, space="PSUM") as psumM,
                    tc.tile_pool(name="wide", bufs=4) as wide,
                    tc.tile_pool(name="p3", bufs=2) as p3,
                ):
                    for c in range(C):
                        AWs, BWs = [], []
                        for g in range(2):
                            for js, dst in ((AJs[g], AWs), (BJs[g], BWs)):
                                Wp = psumU.tile([64, 1024], F32, tag="awp",
                                                name="Wp")
                                for bl in range(2):
                                    jl = js[32 * bl:32 * bl + OB, :].rearrange(
                                        "p (i c) -> p c i", c=3)[:, c, :]
                                    for hh in range(2):
                                        nc.tensor.matmul(
                                            Wp[32 * bl:32 * bl + OB,
                                               hh * 512:hh * 512 + 512],
                                            jl,
                                            wcol[32 * bl:32 * bl + OB,
                                                 hh * 512:hh * 512 + 512],
                                            start=True, stop=True)
                                Ws = wide.tile([64, 1024], F32, tag="aws",
                                               name="Ws")
                                nc.scalar.copy(Ws[:], Wp[:])
                                if dump and c == 0:
                                    dn = "aws" if dst is AWs else "bws"
                                    nc.sync.dma_start(dmp[dn][g], Ws[:])
                                dst.append(Ws)
                        for rc in range(4):
                            OUT = p3.tile([128, 1024], F32, tag="out")
                            for hh in range(2):
                                mAp = psumM.tile([128, 512], F32, tag="map")
                                mBp = psumM.tile([128, 512], F32, tag="map")
                                for cf, (mp, Ws) in enumerate(
                                        ((mAp, AWs), (mBp, BWs))):
                                    for g in range(2):
                                        nc.tensor.matmul(
                                            mp[:],
                                            urow[g][:, rc * 128:(rc + 1) * 128],
                                            Ws[g][:, hh * 512:hh * 512 + 512],
                                            start=(g == 0), stop=(g == 1))
                                PRD = p3.tile([128, 512], F32, tag="prd")
                                nc.vector.tensor_mul(
                                    PRD[:], mAp[:],
                                    G[:, (c * NT + rc) * 1024 + hh * 512:
                                         (c * NT + rc) * 1024 + hh * 512 + 512])
                                MBS = p3.tile([128, 512], F32, tag="mbs")
                                nc.vector.tensor_copy(MBS[:], mBp[:])
                                if dump and c == 0 and rc == 0 and hh == 0:
                                    nc.sync.dma_start(dmp["prd"][:], PRD[:])
                                    nc.sync.dma_start(dmp["mbs"][:], MBS[:])
                                nc.gpsimd.tensor_add(
                                    OUT[:, hh * 512:hh * 512 + 512],
                                    PRD[:], MBS[:])
                            nc.sync.dma_start(
                                out_d[c, rc * 128:(rc + 1) * 128, :], OUT[:])

    nc.compile()
    return nc


_NC_CACHE = {}


def _host_consts():
    # bsum[k, 16*t + m] = 1/4096 if m == 4*t + k//32 (row-block sums, acc over t)
    bsum = np.zeros((128, 64), np.float32)
    for t in range(4):
        for k in range(128):
            bsum[k, 16 * t + 4 * t + k // 32] = 1.0 / 4096.0
    pairm = np.zeros((32, OB), np.float32)
    for i in range(OB):
        pairm[i, i] = 1.0
        pairm[i + 1, i] = 1.0

    def interp(n_out):
        xs = np.linspace(0.0, float(OB - 1), n_out).astype(np.float32)
        x0 = np.floor(xs).astype(np.int32)
        x1 = np.minimum(x0 + 1, OB - 1)
        wx = (xs - x0.astype(np.float32)).astype(np.float32)
        M = np.zeros((OB, n_out), np.float32)
        for j in range(n_out):
            M[x0[j], j] += 1.0 - wx[j]
            M[x1[j], j] += wx[j]
        return M

    return bsum, pairm, interp(W), interp(H)


def kernel(**inputs):
    guide = np.ascontiguousarray(np.asarray(inputs["guide"], dtype=np.float32))
    src = np.ascontiguousarray(np.asarray(inputs["src"], dtype=np.float32))
    w1 = np.asarray(inputs["w1"], dtype=np.float32)
    w2 = np.asarray(inputs["w2"], dtype=np.float32)
    w3 = np.asarray(inputs["w3"], dtype=np.float32)
    g1 = np.asarray(inputs["g1"], dtype=np.float32)
    b1 = np.asarray(inputs["b1"], dtype=np.float32)
    g2 = np.asarray(inputs["g2"], dtype=np.float32)
    b2 = np.asarray(inputs["b2"], dtype=np.float32)

    if "nc" not in _NC_CACHE:
        _NC_CACHE["nc"] = _build_nc()
    nc = _NC_CACHE["nc"]

    bsum, pairm, wcol_b, urow_full = _host_consts()
    wcol = np.zeros((64, W), np.float32)
    wcol[0:OB] = wcol_b
    wcol[32:32 + OB] = wcol_b
    w1ct = np.zeros((64, 32), np.float32)
    w1ct[0:3] = w1[:, 0:3].T
    w1ct[32:35] = w1[:, 0:3].T
    w1vt = np.zeros((64, 32), np.float32)
    w1vt[0:3] = w1[:, 3:6].T
    w1vt[32:35] = w1[:, 3:6].T
    w2t = np.ascontiguousarray(w2.T)
    w3t = np.ascontiguousarray(w3.T)
    bn = np.stack([g1, b1, g2, b2, np.full(32, EPS, np.float32)],
                 axis=1).astype(np.float32)

    in_maps = []
    for k in range(NCORES):
        b, hh = k // 2, k % 2
        ur = [np.zeros((64, SH), np.float32) for _ in range(2)]
        ur[b // 2][32 * (b % 2):32 * (b % 2) + OB, :] = \
            urow_full[:, SH * hh:SH * (hh + 1)]
        in_maps.append(dict(
            g_sh=np.ascontiguousarray(guide[b, :, SH * hh:SH * (hh + 1), :]),
            s_sh=np.ascontiguousarray(src[b, :, SH * hh:SH * (hh + 1), :]),
            bsum=bsum, pairm=pairm, wcol=wcol, urow0=ur[0], urow1=ur[1],
            w1ct=w1ct, w1vt=w1vt, w2t=w2t, w3t=w3t, bn=bn,
        ))

    res = run_bass_kernel_spmd(nc, in_maps, list(range(NCORES)))
    out = np.empty((B, C, H, W), np.float32)
    for k in range(NCORES):
        b, hh = k // 2, k % 2
        out[b, :, SH * hh:SH * (hh + 1), :] = res.results[k]["out_sh"]
    return out



# revision 9
# speedup vs baseline: 1.8788x; 1.8788x over previous
"""ConvGuidedFilter Trainium2 kernel (8-core SPMD, data parallel over (image, row-half)).

Shapes hardcoded for guide/src [4,3,1024,1024] f32, RADIUS=64, STRIDE=32 (box kernel
of ones). Each core handles one (image b, row-half h) shard: [3, 512, 1024].

v2: all PE matmuls run in bf16 (1 cyc/col vs fp32's 4) and phase 3 only
upsamples the core's own image (selected on-chip via a per-core one-hot
selector matmul), instead of computing all four images and multiplying by a
zero-padded row-interp matrix.

Per core:
  P1: load shard; cast guide to resident bf16 (ACT), src to bf16 (DVE);
      products g*s (DVE, bf16 out), g*g (ACT Square, bf16 out); 32-row block
      sums via bf16 PE matmuls (lhsT = 1/4096 block indicators, accumulated
      over the 4 row tiles) into PSUM [64, 1024] pairs; column-block reduce
      (DVE, segments of 32) -> [64,32]; col pair-sum -> [64,31].
  CC: AllGather of the per-core 5952-float stats payload across all 8 cores.
  P2 (replicated): assemble stats, row pair-sum, cov/var (bf16 out), 1x1-conv
      MLP with exact global-batch BatchNorm stats, A for all images in a
      [124=(b,jc), 93=(jr,c)] psum via 31 wide matmuls; box means via pairm
      matmuls; B = meanY - A*meanX; one selector matmul extracts the core's
      own image -> ABsel [31 jc, 186=(A/B, jr, c)] bf16.
  P3: col upsample (bf16 matmuls vs wcol) -> WsAB [64,1024] (A rows 0-30,
      B rows 32-62); row upsample via urowA/urowB [64,128] slices (per-core
      dense interp weights for this half); out = meanA*guide + meanB on DVE;
      DMA out.
"""

import numpy as np
import ml_dtypes

import concourse.bass as bass
import concourse.bacc as bacc
import concourse.mybir as mybir
import concourse.tile as tile
from concourse.bass_utils import run_bass_kernel_spmd

AF = mybir.ActivationFunctionType
ALU = mybir.AluOpType
AX = mybir.AxisListType
F32 = mybir.dt.float32
BF16 = mybir.dt.bfloat16

B, C, H, W = 4, 3, 1024, 1024
NCORES = 8
SH = H // 2          # 512 rows per shard
NT = SH // 128       # 4 row tiles per channel
OB = 31              # box output spatial size
RBL = 16             # row blocks per shard (512/32)
PIX = OB * OB        # 961
NPIX = B * PIX       # 3844
EPS = 1e-5
PAYLOAD = C * 4 * RBL * OB   # 5952
BJC = B * OB         # 124 partitions: (image, box col)


def _build_nc(dump=False):
    nc = bacc.Bacc("TRN2", target_bir_lowering=False, debug=False, num_devices=NCORES)

    g_d = nc.dram_tensor("g_sh", [C, SH, W], F32, kind="ExternalInput")
    s_d = nc.dram_tensor("s_sh", [C, SH, W], F32, kind="ExternalInput")
    bsum_d = nc.dram_tensor("bsum", [128, 64], BF16, kind="ExternalInput")
    pair_d = nc.dram_tensor("pairm", [32, OB], BF16, kind="ExternalInput")
    wcol_d = nc.dram_tensor("wcol", [OB, W], BF16, kind="ExternalInput")
    urowa_d = nc.dram_tensor("urowa", [64, SH], BF16, kind="ExternalInput")
    urowb_d = nc.dram_tensor("urowb", [64, SH], BF16, kind="ExternalInput")
    sel_d = nc.dram_tensor("sel", [BJC, OB], BF16, kind="ExternalInput")
    w1ct_d = nc.dram_tensor("w1ct", [64, 32], BF16, kind="ExternalInput")
    w1vt_d = nc.dram_tensor("w1vt", [64, 32], BF16, kind="ExternalInput")
    w2t_d = nc.dram_tensor("w2t", [32, 32], BF16, kind="ExternalInput")
    w3t_d = nc.dram_tensor("w3t", [32, 3], BF16, kind="ExternalInput")
    bn_d = nc.dram_tensor("bn", [32, 5], F32, kind="ExternalInput")  # g1,b1,g2,b2,eps
    out_d = nc.dram_tensor("out_sh", [C, SH, W], F32, kind="ExternalOutput")
    if dump:
        dmp = {
            "stat": nc.dram_tensor("d_stat", [2, 64, 4 * PIX], F32, kind="ExternalOutput"),
            "cov": nc.dram_tensor("d_cov", [2, 64, PIX], F32, kind="ExternalOutput"),
            "var": nc.dram_tensor("d_var", [2, 64, PIX], F32, kind="ExternalOutput"),
            "h1a": nc.dram_tensor("d_h1a", [32, NPIX], F32, kind="ExternalOutput"),
            "h2a": nc.dram_tensor("d_h2a", [32, NPIX], F32, kind="ExternalOutput"),
            "abal": nc.dram_tensor("d_abal", [BJC, 2 * 93], F32, kind="ExternalOutput"),
            "absel": nc.dram_tensor("d_absel", [OB, 2 * 93], F32, kind="ExternalOutput"),
            "ws": nc.dram_tensor("d_ws", [C, 64, W], F32, kind="ExternalOutput"),
        }

    with tile.TileContext(nc) as tc:
        with (
            tc.tile_pool(name="consts", bufs=1) as consts,
            tc.tile_pool(name="gres", bufs=1) as gres,
            tc.tile_pool(name="persist", bufs=1) as persist,
            tc.tile_pool(name="dram", bufs=1, space="DRAM") as dram,
        ):
            def const(name, dram_t, shape, dt=BF16):
                t = consts.tile(shape, dt, tag=name, name=name + "_sb")
                nc.sync.dma_start(t[:], dram_t[:])
                return t

            bsum = const("bsum", bsum_d, [128, 64])
            pairm = const("pairm", pair_d, [32, OB])
            wcol = const("wcol", wcol_d, [OB, W])
            urowa = const("urowa", urowa_d, [64, SH])
            urowb = const("urowb", urowb_d, [64, SH])
            sel = const("sel", sel_d, [BJC, OB])
            w1ct = const("w1ct", w1ct_d, [64, 32])
            w1vt = const("w1vt", w1vt_d, [64, 32])
            w2t = const("w2t", w2t_d, [32, 32])
            w3t = const("w3t", w3t_d, [32, 3])
            bn = const("bn", bn_d, [32, 5], F32)

            # Guide shard resident in f32 for the final DVE multiply.
            # [128, (c,t)*1024]
            G = gres.tile([128, C * NT * 1024], F32)

            PAYF = persist.tile([64, 2 * C * OB], F32)
            cc_in = dram.tile([64, 2 * C * OB], F32)
            cc_out = dram.tile([NCORES, 64, 2 * C * OB], F32)

            # ---------------- Phase 1 ----------------
            # Stats order s: 0=g(mean_x), 1=src(mean_y), 2=g*s, 3=g*g.
            # P tile A holds stats 0,1 at partition bases 0,32; tile B stats 2,3.
            with (
                tc.tile_pool(name="sload", bufs=3) as sload,
                tc.tile_pool(name="sbfp", bufs=3) as sbfp,
                tc.tile_pool(name="prod", bufs=2) as prod,
                tc.tile_pool(name="psum1", bufs=2, space="PSUM") as psum1,
                tc.tile_pool(name="cred", bufs=2) as cred,
            ):
                for c in range(C):
                    P = [psum1.tile([64, 1024], F32, tag=f"p1_{pi}",
                                    name=f"P{c}_{pi}") for pi in range(2)]
                    for t in range(NT):
                        gsl = G[:, (c * NT + t) * 1024:(c * NT + t + 1) * 1024]
                        nc.sync.dma_start(gsl, g_d[c, t * 128:(t + 1) * 128, :])
                        st = sload.tile([128, 1024], F32, tag="st")
                        nc.sync.dma_start(st[:], s_d[c, t * 128:(t + 1) * 128, :])
                        gbf = sbfp.tile([128, 1024], BF16, tag="gbf")
                        nc.scalar.copy(gbf[:], gsl)
                        sbf = sbfp.tile([128, 1024], BF16, tag="sbf")
                        nc.vector.tensor_copy(sbf[:], st[:])
                        gs = prod.tile([128, 1024], BF16, tag="gs")
                        nc.vector.tensor_mul(gs[:], gbf[:], sbf[:])
                        gg = prod.tile([128, 1024], BF16, tag="gg")
                        nc.scalar.activation(gg[:], gbf[:], AF.Square)
                        for s_idx, srct in enumerate((gbf[:], sbf[:], gs[:], gg[:])):
                            pi, sl = s_idx // 2, s_idx % 2
                            for hh in range(2):
                                nc.tensor.matmul(
                                    P[pi][32 * sl:32 * sl + 16,
                                          hh * 512:hh * 512 + 512],
                                    bsum[:, 16 * t:16 * t + 16],
                                    srct[:, hh * 512:hh * 512 + 512],
                                    start=(t == 0), stop=(t == NT - 1),
                                )
                    for pi in range(2):
                        CR = cred.tile([64, 32], F32, tag=f"cr_{pi}")
                        nc.vector.tensor_reduce(
                            CR[:], P[pi][:].rearrange("p (a b) -> p a b", b=32),
                            axis=AX.X, op=ALU.add,
                        )
                        # stat (pi, sl) at partitions 32*sl+r; free pi*93+c*31+j
                        nc.vector.tensor_add(
                            PAYF[:, pi * 3 * OB + c * OB:pi * 3 * OB + (c + 1) * OB],
                            CR[:, 0:OB], CR[:, 1:32])
                nc.sync.dma_start(cc_in[:], PAYF[:])

            nc.gpsimd.collective_compute(
                "AllGather",
                ALU.bypass,
                replica_groups=[list(range(NCORES))],
                ins=[cc_in.opt()],
                outs=[cc_out.opt()],
            )

            # ---------------- Phase 2 ----------------
            with (
                tc.tile_pool(name="p2", bufs=1) as p2,
                tc.tile_pool(name="p2s", bufs=2) as p2s,
                tc.tile_pool(name="small", bufs=2) as small,
            ):
                # SFg: [64, (s,rb_g,j)]; image 2g+bl channels at base 32*bl
                SF = [p2.tile([64, 4 * 32 * OB], F32, name=f"SF{g}", tag="big",
                              bufs=3) for g in range(2)]
                for g in range(2):
                    for hh in range(2):
                        for pi in range(2):
                            for sl in range(2):
                                s = 2 * pi + sl
                                for bl in range(2):
                                    srcap = cc_out[:].rearrange(
                                        "(gg bl h) (ss r) (pp c j) -> gg h pp ss bl c r j",
                                        gg=2, bl=2, ss=2, pp=2, c=3)[
                                        g, hh, pi, sl, bl][:, 0:RBL, :]
                                    dst = SF[g][:].rearrange(
                                        "(bl cc) (s h r j) -> s h bl cc r j",
                                        bl=2, s=4, h=2, r=RBL)[s, hh, bl, 0:3]
                                    nc.sync.dma_start(dst, srcap)
                # SF2: [rb_g(32), (s2, c, b, j)] f32 then cast to bf16
                # (b,j) contiguous per (s,c) so slices can be matmul lhsT
                SF2 = p2.tile([32, B * 2 * C * OB], F32)
                for hh in range(2):
                    for b in range(B):
                        for si in range(2):
                            srcap = cc_out[:].rearrange(
                                "(bb h) (ss r) (pp c j) -> bb h pp ss r c j",
                                bb=4, ss=2, pp=2, c=3)[b, hh, 0, si][0:RBL]
                            dst = SF2[16 * hh:16 * hh + RBL, :].rearrange(
                                "r (s c b j) -> r s b c j", s=2, c=3, b=B)[:, si, b]
                            nc.sync.dma_start(dst, srcap)
                SF2B = p2.tile([32, B * 2 * C * OB], BF16)
                nc.vector.tensor_copy(SF2B[:], SF2[:])

                STAT, COV, VAR = [], [], []
                for g in range(2):
                    eng = nc.vector if g == 0 else nc.gpsimd
                    ST = p2.tile([64, 4 * PIX], F32, name=f"STAT{g}", tag="big",
                                 bufs=3)
                    sfr = SF[g][:].rearrange("p (s r j) -> p s r j", s=4, r=32)
                    eng.tensor_add(
                        ST[:].rearrange("p (s i j) -> p s i j", s=4, i=OB),
                        sfr[:, :, 0:OB, :], sfr[:, :, 1:32, :])
                    CVg = p2.tile([64, PIX], BF16, name=f"COV{g}", tag="mid", bufs=5)
                    VRg = p2.tile([64, PIX], BF16, name=f"VAR{g}", tag="mid", bufs=5)
                    TMg = p2.tile([64, PIX], F32, name=f"TMP{g}", tag="midf", bufs=3)
                    eng.tensor_mul(TMg[:], ST[:, 0:PIX], ST[:, PIX:2 * PIX])
                    eng.tensor_sub(CVg[:], ST[:, 2 * PIX:3 * PIX], TMg[:])
                    eng.tensor_mul(TMg[:], ST[:, 0:PIX], ST[:, 0:PIX])
                    eng.tensor_sub(VRg[:], ST[:, 3 * PIX:4 * PIX], TMg[:])
                    STAT.append(ST); COV.append(CVg); VAR.append(VRg)
                    if dump:
                        nc.sync.dma_start(dmp["stat"][g], ST[:])
                        nc.vector.tensor_copy(TMg[:], CVg[:])
                        nc.sync.dma_start(dmp["cov"][g], TMg[:])
                        nc.vector.tensor_copy(TMg[:], VRg[:])
                        nc.sync.dma_start(dmp["var"][g], TMg[:])

                chunks = [(0, 512), (512, PIX)]

                def bn_combine(SU, SQ, scn, tbn, gcol, bcol):
                    st_ = small.tile([32, 1], F32, tag="st_" + scn)
                    qt_ = small.tile([32, 1], F32, tag="qt_" + scn)
                    nc.vector.tensor_reduce(st_[:], SU[:], axis=AX.X, op=ALU.add)
                    nc.vector.tensor_reduce(qt_[:], SQ[:], axis=AX.X, op=ALU.add)
                    m_ = small.tile([32, 1], F32, tag="m_" + scn)
                    nc.vector.tensor_scalar_mul(m_[:], st_[:], 1.0 / NPIX)
                    v_ = small.tile([32, 1], F32, tag="v_" + scn)
                    nc.vector.tensor_scalar_mul(v_[:], qt_[:], 1.0 / NPIX)
                    mm_ = small.tile([32, 1], F32, tag="mm_" + scn)
                    nc.vector.tensor_mul(mm_[:], m_[:], m_[:])
                    nc.vector.tensor_sub(v_[:], v_[:], mm_[:])
                    sd_ = small.tile([32, 1], F32, tag="sd_" + scn)
                    nc.scalar.activation(sd_[:], v_[:], AF.Sqrt, bias=bn[:, 4:5])
                    rs_ = small.tile([32, 1], F32, tag="rs_" + scn)
                    nc.vector.reciprocal(rs_[:], sd_[:])
                    sc_ = small.tile([32, 1], F32, tag=scn)
                    nc.vector.tensor_mul(sc_[:], rs_[:], bn[:, gcol:gcol + 1])
                    tb_ = small.tile([32, 1], F32, tag=tbn)
                    nc.vector.tensor_mul(mm_[:], m_[:], sc_[:])
                    nc.vector.tensor_sub(tb_[:], bn[:, bcol:bcol + 1], mm_[:])
                    return sc_, tb_

                with tc.tile_pool(name="psumH", bufs=1, space="PSUM") as psumH:
                    # Layer 1
                    H1p = [psumH.tile([32, PIX], F32, tag=f"h_{b}", name=f"H1p{b}")
                           for b in range(B)]
                    S1 = p2s.tile([32, B], F32, tag="S1")
                    Q1 = p2s.tile([32, B], F32, tag="Q1")
                    SQs = p2s.tile([32, PIX], F32, tag="sqs")
                    for b in range(B):
                        g, bl = b // 2, b % 2
                        for (o, e) in chunks:
                            nc.tensor.matmul(H1p[b][:, o:e],
                                             w1ct[32 * bl:32 * bl + 3, :],
                                             COV[g][32 * bl:32 * bl + 3, o:e],
                                             start=True, stop=False)
                            nc.tensor.matmul(H1p[b][:, o:e],
                                             w1vt[32 * bl:32 * bl + 3, :],
                                             VAR[g][32 * bl:32 * bl + 3, o:e],
                                             start=False, stop=True)
                        nc.vector.tensor_reduce(S1[:, b:b + 1], H1p[b][:],
                                                axis=AX.X, op=ALU.add)
                        nc.scalar.activation(SQs[:], H1p[b][:], AF.Square,
                                             accum_out=Q1[:, b:b + 1])
                    sc1, tb1 = bn_combine(S1, Q1, "sc1", "tb1", 0, 1)
                    H1A = p2.tile([32, NPIX], BF16, tag="bigh", bufs=2)
                    for b in range(B):
                        nc.scalar.activation(H1A[:, b * PIX:(b + 1) * PIX],
                                             H1p[b][:], AF.Relu,
                                             bias=tb1[:], scale=sc1[:])

                    # Layer 2
                    H2p = [psumH.tile([32, PIX], F32, tag=f"h_{b}", name=f"H2p{b}")
                           for b in range(B)]
                    S2 = p2s.tile([32, B], F32, tag="S2")
                    Q2 = p2s.tile([32, B], F32, tag="Q2")
                    for b in range(B):
                        for (o, e) in chunks:
                            nc.tensor.matmul(H2p[b][:, o:e], w2t[:],
                                             H1A[:, b * PIX + o:b * PIX + e],
                                             start=True, stop=True)
                        nc.vector.tensor_reduce(S2[:, b:b + 1], H2p[b][:],
                                                axis=AX.X, op=ALU.add)
                        nc.scalar.activation(SQs[:], H2p[b][:], AF.Square,
                                             accum_out=Q2[:, b:b + 1])
                    sc2, tb2 = bn_combine(S2, Q2, "sc2", "tb2", 2, 3)
                    # H2A free layout (jr, b, jc): the AJall matmuls then read
                    # contiguous [32, 124] lhsT slices per jr.
                    H2A = p2.tile([32, NPIX], BF16, tag="bigh", bufs=2)
                    h2w = H2A[:].rearrange("p (i b j) -> p b i j", i=OB, b=B)
                    for b in range(B):
                        nc.scalar.activation(h2w[:, b],
                                             H2p[b][:].rearrange(
                                                 "p (i j) -> p i j", i=OB),
                                             AF.Relu,
                                             bias=tb2[:], scale=sc2[:])

                if dump:
                    DH = p2s.tile([32, NPIX], F32, tag="dh")
                    nc.vector.tensor_copy(DH[:], H1A[:])
                    nc.sync.dma_start(dmp["h1a"][:], DH[:])
                    nc.vector.tensor_copy(DH[:], H2A[:])
                    nc.sync.dma_start(dmp["h2a"][:], DH[:])

                # A for all 4 images: AJall [124=(b,jc), 93=(jr,c)] psum.
                # lhsT = H2A[:, (b, jr, jc)] sliced at jr -> [32, 4, 31].
                with tc.tile_pool(name="psumA", bufs=1, space="PSUM") as psumA:
                    AJall = psumA.tile([BJC, 3 * OB], F32, tag="ajall")
                    for jr in range(OB):
                        nc.tensor.matmul(AJall[:, 3 * jr:3 * jr + 3],
                                         H2A[:, jr * BJC:(jr + 1) * BJC],
                                         w3t[:],
                                         start=True, stop=True)
                    # Box means of stats 0 (mean_x) and 1 (mean_y):
                    # MXp/MYp [124, 93] via row-block pair sums (pairm).
                    MXp = psumA.tile([BJC, 3 * OB], F32, tag="mxp")
                    MYp = psumA.tile([BJC, 3 * OB], F32, tag="myp")
                    for si, MP in ((0, MXp), (1, MYp)):
                        for cch in range(C):
                            nc.tensor.matmul(
                                MP[:].rearrange("p (i c) -> p c i", c=3)[:, cch, :],
                                SF2B[:, (si * 3 + cch) * BJC:
                                     (si * 3 + cch + 1) * BJC],
                                pairm[:],
                                start=True, stop=True)
                    # ABall [124, 186] bf16: A | B = meanY - A*meanX
                    ABall = p2s.tile([BJC, 2 * 3 * OB], BF16, tag="aball")
                    nc.scalar.copy(ABall[:, 0:3 * OB], AJall[:])
                    MXs = p2s.tile([BJC, 3 * OB], F32, tag="mxs")
                    nc.scalar.copy(MXs[:], MXp[:])
                    TMB = p2s.tile([BJC, 3 * OB], F32, tag="tmb")
                    nc.vector.tensor_mul(TMB[:], AJall[:], MXs[:])
                    nc.vector.tensor_sub(ABall[:, 3 * OB:6 * OB], MYp[:], TMB[:])
                    if dump:
                        DA = p2s.tile([BJC, 2 * 3 * OB], F32, tag="da")
                        nc.vector.tensor_copy(DA[:], ABall[:])
                        nc.sync.dma_start(dmp["abal"][:], DA[:])
                    # Select own image: ABsel [31 jc, 186]
                    ABselp = psumA.tile([OB, 2 * 3 * OB], F32, tag="absel")
                    nc.tensor.matmul(ABselp[:], sel[:], ABall[:],
                                     start=True, stop=True)
                    ABsel = p2s.tile([OB, 2 * 3 * OB], BF16, tag="abselb")
                    nc.scalar.copy(ABsel[:], ABselp[:])
                    if dump:
                        DS = p2s.tile([OB, 2 * 3 * OB], F32, tag="ds")
                        nc.vector.tensor_copy(DS[:], ABsel[:])
                        nc.sync.dma_start(dmp["absel"][:], DS[:])

                # ---------------- Phase 3 ----------------
                absr = ABsel[:].rearrange("p (a i c) -> p a c i", a=2, c=3)
                with (
                    tc.tile_pool(name="psumW", bufs=2, space="PSUM") as psumW,
                    tc.tile_pool(name="psumM", bufs=4, space="PSUM") as psumM,
                    tc.tile_pool(name="wide", bufs=2) as wide,
                    tc.tile_pool(name="p3", bufs=2) as p3,
                ):
                    for c in range(C):
                        Wps = psumW.tile([64, 1024], F32, tag="wps")
                        for ab in range(2):
                            for hh in range(2):
                                nc.tensor.matmul(
                                    Wps[32 * ab:32 * ab + OB,
                                        hh * 512:hh * 512 + 512],
                                    absr[:, ab, c, :],
                                    wcol[:, hh * 512:hh * 512 + 512],
                                    start=True, stop=True)
                        WsAB = wide.tile([64, 1024], BF16, tag="wsab")
                        nc.gpsimd.memset(WsAB[:], 0.0)
                        nc.scalar.copy(WsAB[0:OB, :], Wps[0:OB, :])
                        nc.scalar.copy(WsAB[32:32 + OB, :], Wps[32:32 + OB, :])
                        if dump:
                            DW = p3.tile([64, 1024], F32, tag="dw")
                            nc.vector.tensor_copy(DW[:], WsAB[:])
                            nc.sync.dma_start(dmp["ws"][c], DW[:])
                        for rc in range(4):
                            OUT = p3.tile([128, 1024], F32, tag="out")
                            for hh in range(2):
                                mAp = psumM.tile([128, 512], F32, tag="map")
                                mBp = psumM.tile([128, 512], F32, tag="map")
                                nc.tensor.matmul(
                                    mAp[:], urowa[:, rc * 128:(rc + 1) * 128],
                                    WsAB[:, hh * 512:hh * 512 + 512],
                                    start=True, stop=True)
                                nc.tensor.matmul(
                                    mBp[:], urowb[:, rc * 128:(rc + 1) * 128],
                                    WsAB[:, hh * 512:hh * 512 + 512],
                                    start=True, stop=True)
                                PRD = p3.tile([128, 512], F32, tag="prd")
                                nc.vector.tensor_mul(
                                    PRD[:], mAp[:],
                                    G[:, (c * NT + rc) * 1024 + hh * 512:
                                         (c * NT + rc) * 1024 + hh * 512 + 512])
                                nc.vector.tensor_add(
                                    OUT[:, hh * 512:hh * 512 + 512],
                                    PRD[:], mBp[:])
                            nc.sync.dma_start(
                                out_d[c, rc * 128:(rc + 1) * 128, :], OUT[:])

    nc.compile()
    return nc


_NC_CACHE = {}


def _host_consts():
    # bsum[k, 16*t + m] = 1/4096 if m == 4*t + k//32 (row-block sums, acc over t)
    bsum = np.zeros((128, 64), np.float32)
    for t in range(4):
        for k in range(128):
            bsum[k, 16 * t + 4 * t + k // 32] = 1.0 / 4096.0
    pairm = np.zeros((32, OB), np.float32)
    for i in range(OB):
        pairm[i, i] = 1.0
        pairm[i + 1, i] = 1.0

    def interp(n_out):
        xs = np.linspace(0.0, float(OB - 1), n_out).astype(np.float32)
        x0 = np.floor(xs).astype(np.int32)
        x1 = np.minimum(x0 + 1, OB - 1)
        wx = (xs - x0.astype(np.float32)).astype(np.float32)
        M = np.zeros((OB, n_out), np.float32)
        for j in range(n_out):
            M[x0[j], j] += 1.0 - wx[j]
            M[x1[j], j] += wx[j]
        return M

    return bsum, pairm, interp(W), interp(H)


def _bf(x):
    return np.ascontiguousarray(x.astype(ml_dtypes.bfloat16))


def kernel(**inputs):
    guide = np.ascontiguousarray(np.asarray(inputs["guide"], dtype=np.float32))
    src = np.ascontiguousarray(np.asarray(inputs["src"], dtype=np.float32))
    w1 = np.asarray(inputs["w1"], dtype=np.float32)
    w2 = np.asarray(inputs["w2"], dtype=np.float32)
    w3 = np.asarray(inputs["w3"], dtype=np.float32)
    g1 = np.asarray(inputs["g1"], dtype=np.float32)
    b1 = np.asarray(inputs["b1"], dtype=np.float32)
    g2 = np.asarray(inputs["g2"], dtype=np.float32)
    b2 = np.asarray(inputs["b2"], dtype=np.float32)

    if "nc" not in _NC_CACHE:
        _NC_CACHE["nc"] = _build_nc()
    nc = _NC_CACHE["nc"]

    bsum, pairm, wcol_b, urow_full = _host_consts()
    w1ct = np.zeros((64, 32), np.float32)
    w1ct[0:3] = w1[:, 0:3].T
    w1ct[32:35] = w1[:, 0:3].T
    w1vt = np.zeros((64, 32), np.float32)
    w1vt[0:3] = w1[:, 3:6].T
    w1vt[32:35] = w1[:, 3:6].T
    w2t = np.ascontiguousarray(w2.T)
    w3t = np.ascontiguousarray(w3.T)
    bn = np.stack([g1, b1, g2, b2, np.full(32, EPS, np.float32)],
                  axis=1).astype(np.float32)

    in_maps = []
    for k in range(NCORES):
        b, hh = k // 2, k % 2
        ura = np.zeros((64, SH), np.float32)
        ura[0:OB] = urow_full[:, SH * hh:SH * (hh + 1)]
        urb = np.zeros((64, SH), np.float32)
        urb[32:32 + OB] = urow_full[:, SH * hh:SH * (hh + 1)]
        selm = np.zeros((BJC, OB), np.float32)
        for q in range(OB):
            selm[OB * b + q, q] = 1.0
        in_maps.append(dict(
            g_sh=np.ascontiguousarray(guide[b, :, SH * hh:SH * (hh + 1), :]),
            s_sh=np.ascontiguousarray(src[b, :, SH * hh:SH * (hh + 1), :]),
            bsum=_bf(bsum), pairm=_bf(pairm), wcol=_bf(wcol_b),
            urowa=_bf(ura), urowb=_bf(urb), sel=_bf(selm),
            w1ct=_bf(w1ct), w1vt=_bf(w1vt), w2t=_bf(w2t), w3t=_bf(w3t),
            bn=bn,
        ))

    res = run_bass_kernel_spmd(nc, in_maps, list(range(NCORES)))
    out = np.empty((B, C, H, W), np.float32)
    for k in range(NCORES):
        b, hh = k // 2, k % 2
        out[b, :, SH * hh:SH * (hh + 1), :] = res.results[k]["out_sh"]
    return out


# revision 20
# speedup vs baseline: 1.9295x; 1.0270x over previous
"""ConvGuidedFilter Trainium2 kernel (8-core SPMD, data parallel over (image, row-half)).

Shapes hardcoded for guide/src [4,3,1024,1024] f32, RADIUS=64, STRIDE=32 (box kernel
of ones). Each core handles one (image b, row-half h) shard: [3, 512, 1024].

v2: all PE matmuls run in bf16 (1 cyc/col vs fp32's 4) and phase 3 only
upsamples the core's own image (selected on-chip via a per-core one-hot
selector matmul), instead of computing all four images and multiplying by a
zero-padded row-interp matrix.

Per core:
  P1: load shard; cast guide to resident bf16 (ACT), src to bf16 (DVE);
      products g*s (DVE, bf16 out), g*g (ACT Square, bf16 out); 32-row block
      sums via bf16 PE matmuls (lhsT = 1/4096 block indicators, accumulated
      over the 4 row tiles) into PSUM [64, 1024] pairs; column-block reduce
      (DVE, segments of 32) -> [64,32]; col pair-sum -> [64,31].
  CC: AllGather of the per-core 5952-float stats payload across all 8 cores.
  P2 (replicated): assemble stats, row pair-sum, cov/var (bf16 out), 1x1-conv
      MLP with exact global-batch BatchNorm stats, A for all images in a
      [124=(b,jc), 93=(jr,c)] psum via 31 wide matmuls; box means via pairm
      matmuls; B = meanY - A*meanX; one selector matmul extracts the core's
      own image -> ABsel [31 jc, 186=(A/B, jr, c)] bf16.
  P3: col upsample (bf16 matmuls vs wcol) -> WsAB [64,1024] (A rows 0-30,
      B rows 32-62); row upsample via urowA/urowB [64,128] slices (per-core
      dense interp weights for this half); out = meanA*guide + meanB on DVE;
      DMA out.
"""

import numpy as np
import ml_dtypes

import concourse.bass as bass
import concourse.bacc as bacc
import concourse.mybir as mybir
import concourse.tile as tile
from concourse.bass_utils import run_bass_kernel_spmd

AF = mybir.ActivationFunctionType
ALU = mybir.AluOpType
AX = mybir.AxisListType
F32 = mybir.dt.float32
BF16 = mybir.dt.bfloat16

B, C, H, W = 4, 3, 1024, 1024
NCORES = 8
SH = H // 2          # 512 rows per shard
NT = SH // 128       # 4 row tiles per channel
OB = 31              # box output spatial size
RBL = 16             # row blocks per shard (512/32)
PIX = OB * OB        # 961
NPIX = B * PIX       # 3844
EPS = 1e-5
PAYLOAD = C * 4 * RBL * OB   # 5952
BJC = B * OB         # 124 partitions: (image, box col)


def _build_nc(dump=False):
    nc = bacc.Bacc("TRN2", target_bir_lowering=False, debug=False, num_devices=NCORES)

    g_d = nc.dram_tensor("g_sh", [C, SH, W], F32, kind="ExternalInput")
    s_d = nc.dram_tensor("s_sh", [C, SH, W], F32, kind="ExternalInput")
    bsum_d = nc.dram_tensor("bsum", [128, 64], BF16, kind="ExternalInput")
    pair_d = nc.dram_tensor("pairm", [32, OB], BF16, kind="ExternalInput")
    wcol_d = nc.dram_tensor("wcol", [OB, W], BF16, kind="ExternalInput")
    urowa_d = nc.dram_tensor("urowa", [64, SH], BF16, kind="ExternalInput")
    urowb_d = nc.dram_tensor("urowb", [64, SH], BF16, kind="ExternalInput")
    sel_d = nc.dram_tensor("sel", [BJC, OB], BF16, kind="ExternalInput")
    w1ct_d = nc.dram_tensor("w1ct", [64, 32], BF16, kind="ExternalInput")
    w1vt_d = nc.dram_tensor("w1vt", [64, 32], BF16, kind="ExternalInput")
    w2t_d = nc.dram_tensor("w2t", [32, 32], BF16, kind="ExternalInput")
    w3t_d = nc.dram_tensor("w3t", [32, 3], BF16, kind="ExternalInput")
    bn_d = nc.dram_tensor("bn", [32, 5], F32, kind="ExternalInput")  # g1,b1,g2,b2,eps
    out_d = nc.dram_tensor("out_sh", [C, SH, W], F32, kind="ExternalOutput")
    if dump:
        dmp = {
            "stat": nc.dram_tensor("d_stat", [2, 64, 4 * PIX], F32, kind="ExternalOutput"),
            "cov": nc.dram_tensor("d_cov", [2, 64, PIX], F32, kind="ExternalOutput"),
            "var": nc.dram_tensor("d_var", [2, 64, PIX], F32, kind="ExternalOutput"),
            "h1a": nc.dram_tensor("d_h1a", [32, NPIX], F32, kind="ExternalOutput"),
            "h2a": nc.dram_tensor("d_h2a", [32, NPIX], F32, kind="ExternalOutput"),
            "abal": nc.dram_tensor("d_abal", [BJC, 2 * 93], F32, kind="ExternalOutput"),
            "absel": nc.dram_tensor("d_absel", [OB, 2 * 93], F32, kind="ExternalOutput"),
            "ws": nc.dram_tensor("d_ws", [C, 64, W], F32, kind="ExternalOutput"),
        }

    with tile.TileContext(nc) as tc:
        with (
            tc.tile_pool(name="consts", bufs=1) as consts,
            tc.tile_pool(name="gres", bufs=1) as gres,
            tc.tile_pool(name="persist", bufs=1) as persist,
            tc.tile_pool(name="dram", bufs=1, space="DRAM") as dram,
        ):
            def const(name, dram_t, shape, dt=BF16):
                t = consts.tile(shape, dt, tag=name, name=name + "_sb")
                nc.sync.dma_start(t[:], dram_t[:])
                return t

            bsum = const("bsum", bsum_d, [128, 64])
            pairm = const("pairm", pair_d, [32, OB])
            wcol = const("wcol", wcol_d, [OB, W])
            urowa = const("urowa", urowa_d, [64, SH])
            urowb = const("urowb", urowb_d, [64, SH])
            sel = const("sel", sel_d, [BJC, OB])
            w1ct = const("w1ct", w1ct_d, [64, 32])
            w1vt = const("w1vt", w1vt_d, [64, 32])
            w2t = const("w2t", w2t_d, [32, 32])
            w3t = const("w3t", w3t_d, [32, 3])
            bn = const("bn", bn_d, [32, 5], F32)

            # Guide shard resident in f32 for the final DVE multiply.
            # [128, (c,t)*1024]
            G = gres.tile([128, C * NT * 1024], F32)

            # Per-channel collective payload [64=(sl,rb), 62=(pi,j)]: each
            # channel's stats AllGather overlaps the next channel's compute.
            cc_in = [dram.tile([64, 2 * OB], F32, tag=f"cci{c}",
                               name=f"cc_in{c}") for c in range(C)]
            cc_out = [dram.tile([NCORES, 64, 2 * OB], F32, tag=f"cco{c}",
                                name=f"cc_out{c}") for c in range(C)]

            # ---------------- Phase 1 ----------------
            # Stats order s: 0=g(mean_x), 1=src(mean_y), 2=g*s, 3=g*g.
            # P tile A holds stats 0,1 at partition bases 0,32; tile B stats 2,3.
            with (
                tc.tile_pool(name="sload", bufs=2) as sload,
                tc.tile_pool(name="sbfp", bufs=3) as sbfp,
                tc.tile_pool(name="prod", bufs=2) as prod,
                tc.tile_pool(name="psum1", bufs=2, space="PSUM") as psum1,
                tc.tile_pool(name="cred", bufs=2) as cred,
            ):
                for c in range(C):
                    P = [psum1.tile([64, 1024], F32, tag=f"p1_{pi}",
                                    name=f"P{c}_{pi}") for pi in range(2)]
                    # one DMA per channel: [512,1024] DRAM -> [128,(t,x)] SBUF
                    gc = G[:, c * NT * 1024:(c + 1) * NT * 1024]
                    nc.sync.dma_start(
                        gc.rearrange("p (t x) -> p t x", t=NT),
                        g_d[c].rearrange("(t p) x -> p t x", t=NT))
                    sc = sload.tile([128, NT * 1024], F32, tag="st")
                    nc.sync.dma_start(
                        sc[:].rearrange("p (t x) -> p t x", t=NT),
                        s_d[c].rearrange("(t p) x -> p t x", t=NT))
                    for t in range(NT):
                        gsl = G[:, (c * NT + t) * 1024:(c * NT + t + 1) * 1024]
                        st = sc[:, t * 1024:(t + 1) * 1024]
                        gbf = sbfp.tile([128, 1024], BF16, tag="gbf")
                        nc.scalar.copy(gbf[:], gsl)
                        sbf = sbfp.tile([128, 1024], BF16, tag="sbf")
                        nc.vector.tensor_copy(sbf[:], st)
                        gs = prod.tile([128, 1024], BF16, tag="gs")
                        nc.vector.tensor_mul(gs[:], gbf[:], sbf[:])
                        gg = prod.tile([128, 1024], BF16, tag="gg")
                        nc.scalar.activation(gg[:], gbf[:], AF.Square)
                        for s_idx, srct in enumerate((gbf[:], sbf[:], gs[:], gg[:])):
                            pi, sl = s_idx // 2, s_idx % 2
                            for hh in range(2):
                                nc.tensor.matmul(
                                    P[pi][32 * sl:32 * sl + 16,
                                          hh * 512:hh * 512 + 512],
                                    bsum[:, 16 * t:16 * t + 16],
                                    srct[:, hh * 512:hh * 512 + 512],
                                    start=(t == 0), stop=(t == NT - 1),
                                )
                    PAYC = cred.tile([64, 2 * OB], F32, tag="payc")
                    for pi in range(2):
                        CR = cred.tile([64, 32], F32, tag=f"cr_{pi}")
                        nc.vector.tensor_reduce(
                            CR[:], P[pi][:].rearrange("p (a b) -> p a b", b=32),
                            axis=AX.X, op=ALU.add,
                        )
                        # stat (pi, sl) at partitions 32*sl+r; free pi*31+j
                        nc.vector.tensor_add(
                            PAYC[:, pi * OB:(pi + 1) * OB],
                            CR[:, 0:OB], CR[:, 1:32])
                    nc.sync.dma_start(cc_in[c][:], PAYC[:])
                    nc.gpsimd.collective_compute(
                        "AllGather",
                        ALU.bypass,
                        replica_groups=[list(range(NCORES))],
                        ins=[cc_in[c].opt()],
                        outs=[cc_out[c].opt()],
                    )

            # ---------------- Phase 2 ----------------
            with (
                tc.tile_pool(name="p2", bufs=1) as p2,
                tc.tile_pool(name="p2s", bufs=2) as p2s,
                tc.tile_pool(name="small", bufs=2) as small,
            ):
                # SFg: [64, (s,rb_g,j)]; image 2g+bl channels at base 32*bl
                SF = [p2.tile([64, 4 * 32 * OB], F32, name=f"SF{g}", tag="big",
                              bufs=3) for g in range(2)]
                for g in range(2):
                    for c in range(C):
                        for pi in range(2):
                            for sl in range(2):
                                for bl in range(2):
                                    srcap = cc_out[c][:].rearrange(
                                        "(gg bl h) (ss r) (pp j) -> gg bl ss pp h r j",
                                        gg=2, bl=2, ss=2, pp=2)[
                                        g, bl, sl][pi:pi + 1, :, 0:RBL, :]
                                    dst = SF[g][:].rearrange(
                                        "(bl cc) (s h r j) -> bl cc s h r j",
                                        bl=2, s=4, h=2, r=RBL)[
                                        bl, c:c + 1, 2 * pi + sl]
                                    nc.sync.dma_start(dst, srcap)
                # SF2: [rb_g(32), (s2, c, b, j)] f32 then cast to bf16
                # (b,j) contiguous per (s,c) so slices can be matmul lhsT
                SF2 = p2.tile([32, B * 2 * C * OB], F32)
                for c in range(C):
                    for si in range(2):
                        for hh in range(2):
                            srcap = cc_out[c][:].rearrange(
                                "(bb h) (ss r) (pp j) -> h pp ss r bb j",
                                bb=4, ss=2, pp=2)[hh, 0, si][0:RBL]
                            dst = SF2[16 * hh:16 * hh + RBL, :].rearrange(
                                "r (s c b j) -> r s c b j",
                                s=2, c=3, b=B)[:, si, c]
                            nc.sync.dma_start(dst, srcap)
                SF2B = p2.tile([32, B * 2 * C * OB], BF16)
                nc.vector.tensor_copy(SF2B[:], SF2[:])

                STAT, COV, VAR = [], [], []
                for g in range(2):
                    eng = nc.vector
                    ST = p2.tile([64, 4 * PIX], F32, name=f"STAT{g}", tag="big",
                                 bufs=3)
                    sfr = SF[g][:].rearrange("p (s r j) -> p s r j", s=4, r=32)
                    eng.tensor_add(
                        ST[:].rearrange("p (s i j) -> p s i j", s=4, i=OB),
                        sfr[:, :, 0:OB, :], sfr[:, :, 1:32, :])
                    CVg = p2.tile([64, PIX], BF16, name=f"COV{g}", tag="mid", bufs=5)
                    VRg = p2.tile([64, PIX], BF16, name=f"VAR{g}", tag="mid", bufs=5)
                    TMg = p2.tile([64, PIX], F32, name=f"TMP{g}", tag="midf", bufs=3)
                    eng.tensor_mul(TMg[:], ST[:, 0:PIX], ST[:, PIX:2 * PIX])
                    eng.tensor_sub(CVg[:], ST[:, 2 * PIX:3 * PIX], TMg[:])
                    eng.tensor_mul(TMg[:], ST[:, 0:PIX], ST[:, 0:PIX])
                    eng.tensor_sub(VRg[:], ST[:, 3 * PIX:4 * PIX], TMg[:])
                    STAT.append(ST); COV.append(CVg); VAR.append(VRg)
                    if dump:
                        nc.sync.dma_start(dmp["stat"][g], ST[:])
                        nc.vector.tensor_copy(TMg[:], CVg[:])
                        nc.sync.dma_start(dmp["cov"][g], TMg[:])
                        nc.vector.tensor_copy(TMg[:], VRg[:])
                        nc.sync.dma_start(dmp["var"][g], TMg[:])

                chunks = [(0, 512), (512, PIX)]

                def bn_combine(SU, SQ, scn, tbn, gcol, bcol):
                    st_ = small.tile([32, 1], F32, tag="st_" + scn)
                    qt_ = small.tile([32, 1], F32, tag="qt_" + scn)
                    nc.vector.tensor_reduce(st_[:], SU[:], axis=AX.X, op=ALU.add)
                    nc.vector.tensor_reduce(qt_[:], SQ[:], axis=AX.X, op=ALU.add)
                    m_ = small.tile([32, 1], F32, tag="m_" + scn)
                    nc.vector.tensor_scalar_mul(m_[:], st_[:], 1.0 / NPIX)
                    v_ = small.tile([32, 1], F32, tag="v_" + scn)
                    nc.vector.tensor_scalar_mul(v_[:], qt_[:], 1.0 / NPIX)
                    mm_ = small.tile([32, 1], F32, tag="mm_" + scn)
                    nc.vector.tensor_mul(mm_[:], m_[:], m_[:])
                    nc.vector.tensor_sub(v_[:], v_[:], mm_[:])
                    sd_ = small.tile([32, 1], F32, tag="sd_" + scn)
                    nc.scalar.activation(sd_[:], v_[:], AF.Sqrt, bias=bn[:, 4:5])
                    rs_ = small.tile([32, 1], F32, tag="rs_" + scn)
                    nc.vector.reciprocal(rs_[:], sd_[:])
                    sc_ = small.tile([32, 1], F32, tag=scn)
                    nc.vector.tensor_mul(sc_[:], rs_[:], bn[:, gcol:gcol + 1])
                    tb_ = small.tile([32, 1], F32, tag=tbn)
                    nc.vector.tensor_mul(mm_[:], m_[:], sc_[:])
                    nc.vector.tensor_sub(tb_[:], bn[:, bcol:bcol + 1], mm_[:])
                    return sc_, tb_

                with tc.tile_pool(name="psumH", bufs=1, space="PSUM") as psumH:
                    # Layer 1
                    H1p = [psumH.tile([32, PIX], F32, tag=f"h_{b}", name=f"H1p{b}")
                           for b in range(B)]
                    S1 = p2s.tile([32, B], F32, tag="S1")
                    Q1 = p2s.tile([32, B], F32, tag="Q1")
                    SQs = p2s.tile([32, PIX], F32, tag="sqs")
                    for b in range(B):
                        g, bl = b // 2, b % 2
                        for (o, e) in chunks:
                            nc.tensor.matmul(H1p[b][:, o:e],
                                             w1ct[32 * bl:32 * bl + 3, :],
                                             COV[g][32 * bl:32 * bl + 3, o:e],
                                             start=True, stop=False)
                            nc.tensor.matmul(H1p[b][:, o:e],
                                             w1vt[32 * bl:32 * bl + 3, :],
                                             VAR[g][32 * bl:32 * bl + 3, o:e],
                                             start=False, stop=True)
                        nc.vector.tensor_reduce(S1[:, b:b + 1], H1p[b][:],
                                                axis=AX.X, op=ALU.add)
                        nc.scalar.activation(SQs[:], H1p[b][:], AF.Square,
                                             accum_out=Q1[:, b:b + 1])
                    sc1, tb1 = bn_combine(S1, Q1, "sc1", "tb1", 0, 1)
                    H1A = p2.tile([32, NPIX], BF16, tag="bigh", bufs=2)
                    for b in range(B):
                        nc.scalar.activation(H1A[:, b * PIX:(b + 1) * PIX],
                                             H1p[b][:], AF.Relu,
                                             bias=tb1[:], scale=sc1[:])

                    # Layer 2
                    H2p = [psumH.tile([32, PIX], F32, tag=f"h_{b}", name=f"H2p{b}")
                           for b in range(B)]
                    S2 = p2s.tile([32, B], F32, tag="S2")
                    Q2 = p2s.tile([32, B], F32, tag="Q2")
                    for b in range(B):
                        for (o, e) in chunks:
                            nc.tensor.matmul(H2p[b][:, o:e], w2t[:],
                                             H1A[:, b * PIX + o:b * PIX + e],
                                             start=True, stop=True)
                        nc.vector.tensor_reduce(S2[:, b:b + 1], H2p[b][:],
                                                axis=AX.X, op=ALU.add)
                        nc.scalar.activation(SQs[:], H2p[b][:], AF.Square,
                                             accum_out=Q2[:, b:b + 1])
                    sc2, tb2 = bn_combine(S2, Q2, "sc2", "tb2", 2, 3)
                    # H2A free layout (jr, b, jc): the AJall matmuls then read
                    # contiguous [32, 124] lhsT slices per jr.
                    H2A = p2.tile([32, NPIX], BF16, tag="bigh", bufs=2)
                    h2w = H2A[:].rearrange("p (i b j) -> p b i j", i=OB, b=B)
                    for b in range(B):
                        nc.scalar.activation(h2w[:, b],
                                             H2p[b][:].rearrange(
                                                 "p (i j) -> p i j", i=OB),
                                             AF.Relu,
                                             bias=tb2[:], scale=sc2[:])

                if dump:
                    DH = p2s.tile([32, NPIX], F32, tag="dh")
                    nc.vector.tensor_copy(DH[:], H1A[:])
                    nc.sync.dma_start(dmp["h1a"][:], DH[:])
                    nc.vector.tensor_copy(DH[:], H2A[:])
                    nc.sync.dma_start(dmp["h2a"][:], DH[:])

                # A for all 4 images: AJall [124=(b,jc), 93=(jr,c)] psum.
                # lhsT = H2A[:, (b, jr, jc)] sliced at jr -> [32, 4, 31].
                with tc.tile_pool(name="psumA", bufs=1, space="PSUM") as psumA:
                    AJall = psumA.tile([BJC, 3 * OB], F32, tag="ajall")
                    for jr in range(OB):
                        nc.tensor.matmul(AJall[:, 3 * jr:3 * jr + 3],
                                         H2A[:, jr * BJC:(jr + 1) * BJC],
                                         w3t[:],
                                         start=True, stop=True)
                    # Box means of stats 0 (mean_x) and 1 (mean_y):
                    # MXp/MYp [124, 93] via row-block pair sums (pairm).
                    MXp = psumA.tile([BJC, 3 * OB], F32, tag="mxp")
                    MYp = psumA.tile([BJC, 3 * OB], F32, tag="myp")
                    for si, MP in ((0, MXp), (1, MYp)):
                        for cch in range(C):
                            nc.tensor.matmul(
                                MP[:].rearrange("p (i c) -> p c i", c=3)[:, cch, :],
                                SF2B[:, (si * 3 + cch) * BJC:
                                     (si * 3 + cch + 1) * BJC],
                                pairm[:],
                                start=True, stop=True)
                    # ABall [124, 186] bf16: A | B = meanY - A*meanX
                    ABall = p2s.tile([BJC, 2 * 3 * OB], BF16, tag="aball")
                    nc.scalar.copy(ABall[:, 0:3 * OB], AJall[:])
                    MXs = p2s.tile([BJC, 3 * OB], F32, tag="mxs")
                    nc.scalar.copy(MXs[:], MXp[:])
                    TMB = p2s.tile([BJC, 3 * OB], F32, tag="tmb")
                    nc.vector.tensor_mul(TMB[:], AJall[:], MXs[:])
                    nc.vector.tensor_sub(ABall[:, 3 * OB:6 * OB], MYp[:], TMB[:])
                    if dump:
                        DA = p2s.tile([BJC, 2 * 3 * OB], F32, tag="da")
                        nc.vector.tensor_copy(DA[:], ABall[:])
                        nc.sync.dma_start(dmp["abal"][:], DA[:])
                    # Select own image: ABsel [31 jc, 186]
                    ABselp = psumA.tile([OB, 2 * 3 * OB], F32, tag="absel")
                    nc.tensor.matmul(ABselp[:], sel[:], ABall[:],
                                     start=True, stop=True)
                    ABsel = p2s.tile([OB, 2 * 3 * OB], BF16, tag="abselb")
                    nc.scalar.copy(ABsel[:], ABselp[:])
                    if dump:
                        DS = p2s.tile([OB, 2 * 3 * OB], F32, tag="ds")
                        nc.vector.tensor_copy(DS[:], ABsel[:])
                        nc.sync.dma_start(dmp["absel"][:], DS[:])

                # ---------------- Phase 3 ----------------
                absr = ABsel[:].rearrange("p (a i c) -> p a c i", a=2, c=3)
                with (
                    tc.tile_pool(name="psumW", bufs=2, space="PSUM") as psumW,
                    tc.tile_pool(name="psumM", bufs=4, space="PSUM") as psumM,
                    tc.tile_pool(name="wide", bufs=2) as wide,
                    tc.tile_pool(name="p3", bufs=2) as p3,
                ):
                    for c in range(C):
                        Wps = psumW.tile([64, 1024], F32, tag="wps")
                        for ab in range(2):
                            for hh in range(2):
                                nc.tensor.matmul(
                                    Wps[32 * ab:32 * ab + OB,
                                        hh * 512:hh * 512 + 512],
                                    absr[:, ab, c, :],
                                    wcol[:, hh * 512:hh * 512 + 512],
                                    start=True, stop=True)
                        WsAB = wide.tile([64, 1024], BF16, tag="wsab")
                        nc.gpsimd.memset(WsAB[:], 0.0)
                        nc.scalar.copy(WsAB[0:OB, :], Wps[0:OB, :])
                        nc.scalar.copy(WsAB[32:32 + OB, :], Wps[32:32 + OB, :])
                        if dump:
                            DW = p3.tile([64, 1024], F32, tag="dw")
                            nc.vector.tensor_copy(DW[:], WsAB[:])
                            nc.sync.dma_start(dmp["ws"][c], DW[:])
                        OUT = p3.tile([128, NT * 1024], F32, tag="out")
                        for rc in range(4):
                            for hh in range(2):
                                mAp = psumM.tile([128, 512], F32, tag="map")
                                mBp = psumM.tile([128, 512], F32, tag="map")
                                nc.tensor.matmul(
                                    mAp[:], urowa[:, rc * 128:(rc + 1) * 128],
                                    WsAB[:, hh * 512:hh * 512 + 512],
                                    start=True, stop=True)
                                nc.tensor.matmul(
                                    mBp[:], urowb[:, rc * 128:(rc + 1) * 128],
                                    WsAB[:, hh * 512:hh * 512 + 512],
                                    start=True, stop=True)
                                PRD = p3.tile([128, 512], F32, tag="prd")
                                nc.vector.tensor_mul(
                                    PRD[:], mAp[:],
                                    G[:, (c * NT + rc) * 1024 + hh * 512:
                                         (c * NT + rc) * 1024 + hh * 512 + 512])
                                nc.vector.tensor_add(
                                    OUT[:, rc * 1024 + hh * 512:
                                        rc * 1024 + hh * 512 + 512],
                                    PRD[:], mBp[:])
                        # one store per channel: [128,(rc,x)] -> [512,1024]
                        nc.sync.dma_start(
                            out_d[c].rearrange("(t p) x -> p t x", t=NT),
                            OUT[:].rearrange("p (t x) -> p t x", t=NT))

    nc.compile()
    return nc


_NC_CACHE = {}


def _host_consts():
    # bsum[k, 16*t + m] = 1/4096 if m == 4*t + k//32 (row-block sums, acc over t)
    bsum = np.zeros((128, 64), np.float32)
    for t in range(4):
        for k in range(128):
            bsum[k, 16 * t + 4 * t + k // 32] = 1.0 / 4096.0
    pairm = np.zeros((32, OB), np.float32)
    for i in range(OB):
        pairm[i, i] = 1.0
        pairm[i + 1, i] = 1.0

    def interp(n_out):
        xs = np.linspace(0.0, float(OB - 1), n_out).astype(np.float32)
        x0 = np.floor(xs).astype(np.int32)
        x1 = np.minimum(x0 + 1, OB - 1)
        wx = (xs - x0.astype(np.float32)).astype(np.float32)
        M = np.zeros((OB, n_out), np.float32)
        for j in range(n_out):
            M[x0[j], j] += 1.0 - wx[j]
            M[x1[j], j] += wx[j]
        return M

    return bsum, pairm, interp(W), interp(H)


def _bf(x):
    return np.ascontiguousarray(x.astype(ml_dtypes.bfloat16))


def kernel(**inputs):
    guide = np.ascontiguousarray(np.asarray(inputs["guide"], dtype=np.float32))
    src = np.ascontiguousarray(np.asarray(inputs["src"], dtype=np.float32))
    w1 = np.asarray(inputs["w1"], dtype=np.float32)
    w2 = np.asarray(inputs["w2"], dtype=np.float32)
    w3 = np.asarray(inputs["w3"], dtype=np.float32)
    g1 = np.asarray(inputs["g1"], dtype=np.float32)
    b1 = np.asarray(inputs["b1"], dtype=np.float32)
    g2 = np.asarray(inputs["g2"], dtype=np.float32)
    b2 = np.asarray(inputs["b2"], dtype=np.float32)

    if "nc" not in _NC_CACHE:
        _NC_CACHE["nc"] = _build_nc()
    nc = _NC_CACHE["nc"]

    bsum, pairm, wcol_b, urow_full = _host_consts()
    w1ct = np.zeros((64, 32), np.float32)
    w1ct[0:3] = w1[:, 0:3].T
    w1ct[32:35] = w1[:, 0:3].T
    w1vt = np.zeros((64, 32), np.float32)
    w1vt[0:3] = w1[:, 3:6].T
    w1vt[32:35] = w1[:, 3:6].T
    w2t = np.ascontiguousarray(w2.T)
    w3t = np.ascontiguousarray(w3.T)
    bn = np.stack([g1, b1, g2, b2, np.full(32, EPS, np.float32)],
                  axis=1).astype(np.float32)

    in_maps = []
    for k in range(NCORES):
        b, hh = k // 2, k % 2
        ura = np.zeros((64, SH), np.float32)
        ura[0:OB] = urow_full[:, SH * hh:SH * (hh + 1)]
        urb = np.zeros((64, SH), np.float32)
        urb[32:32 + OB] = urow_full[:, SH * hh:SH * (hh + 1)]
        selm = np.zeros((BJC, OB), np.float32)
        for q in range(OB):
            selm[OB * b + q, q] = 1.0
        in_maps.append(dict(
            g_sh=np.ascontiguousarray(guide[b, :, SH * hh:SH * (hh + 1), :]),
            s_sh=np.ascontiguousarray(src[b, :, SH * hh:SH * (hh + 1), :]),
            bsum=_bf(bsum), pairm=_bf(pairm), wcol=_bf(wcol_b),
            urowa=_bf(ura), urowb=_bf(urb), sel=_bf(selm),
            w1ct=_bf(w1ct), w1vt=_bf(w1vt), w2t=_bf(w2t), w3t=_bf(w3t),
            bn=bn,
        ))

    res = run_bass_kernel_spmd(nc, in_maps, list(range(NCORES)))
    out = np.empty((B, C, H, W), np.float32)
    for k in range(NCORES):
        b, hh = k // 2, k % 2
        out[b, :, SH * hh:SH * (hh + 1), :] = res.results[k]["out_sh"]
    return out
